# revision 10
# baseline (speedup 1.0000x reference)
"""Trainium2 Bass kernel for nn_FDCKernel (fused phase / cos + eta reduction).

Reference computation:
    theta    = x @ h.T + delta          # [B, 256], B = 262144
    out_real = cos(theta)               # returned, 256 MiB -> memory bound
    eta      = sum(arctan2(mean_b sin(theta), mean_b cos(theta))) / pi

Strategy:
  * Pure data parallelism over the batch across 8 NeuronCores.
  * Per core, theta_c = theta + pi/2 comes from ONE PE matmul per 128-row
    tile.  fp32 matmuls on TRN2 run as 2x(LDWEIGHTS+MATMUL) LOW/HIGH passes
    (~854 ns each), so instead the K=3 fp32 contraction is decomposed into a
    K=15 all-bf16 contraction (3-term bf16 splits of x, h and delta+pi/2,
    keeping all cross products >= 2^-27): single-pass bf16 matmul + FWL
    weight loads, ~4-6x faster on PE, while PSUM still accumulates in fp32
    so theta keeps ~1e-7 accuracy.
  * ACT engine turns [128, 2048] PSUM tiles into cos via Sin(theta + pi/2)
    in one transcendental pass.
  * The host pre-permutes the batch within every 1024-row block (col w*128+p
    holds row p*8+w) so each SBUF partition owns 8 consecutive output rows:
    DMA stores are 1 MiB per dma_start with 8 KiB contiguous descriptors.
  * eta needs only the per-column MEANS of sin/cos.  theta = u*p + v*q + d
    with |theta| << 1, so mean_b cos/sin expand exactly into polynomial
    moments S_ij = sum_b u^i v^j of the tiny [B,2] input x, evaluated on the
    host in float64 (Taylor to order 20) - no device sin pass, no collective.
"""

import math
import sys

import ml_dtypes
import numpy as np

sys.path.insert(0, "/opt/trn_rl_repo")

import concourse.bass as bass
import concourse.tile as tile
from concourse import bacc, mybir
from concourse.bass_utils import run_bass_kernel_spmd

B, IN_DIM, OUT_DIM = 262144, 2, 256
N_CORES = 8
BS = B // N_CORES            # 32768 rows per core
P = 128                      # batch rows per partition tile
W = 8                        # batch subtiles per PSUM tile (free dim = W*256 = 2048)
ROWS_PER_IT = P * W          # 1024
N_IT = BS // ROWS_PER_IT     # 32
FREE = W * OUT_DIM           # 2048
K = 15                       # bf16 contraction depth (see _build_lhsT/_build_rhs)

BF16 = ml_dtypes.bfloat16

_cache = {}
LAST_RESULTS = None          # BassKernelResults of the most recent run (for profiling)


def _split3(v64):
    """v (float64/32) ~= t1+t2+t3 with bf16 terms; residual <= 2^-27 |v|."""
    t1 = v64.astype(BF16)
    r1 = (v64 - t1.astype(np.float64)).astype(np.float64)
    t2 = r1.astype(BF16)
    r2 = r1 - t2.astype(np.float64)
    t3 = r2.astype(BF16)
    return t1, t2, t3


def _build_nc():
    f32 = mybir.dt.float32
    bf16 = mybir.dt.bfloat16
    # Bacc (not raw Bass): its compile() runs the TRN2 legalization passes —
    # move_matmul_waits_to_ldweights + generate_event_semaphores (1-wait limit).
    nc = bacc.Bacc(
        "TRN2", target_bir_lowering=False, debug=False, num_devices=N_CORES
    )
    xt = nc.declare_dram_parameter("xt", [K, BS], bf16, isOutput=False)
    rhs = nc.declare_dram_parameter("rhs", [K, OUT_DIM], bf16, isOutput=False)
    out = nc.declare_dram_parameter("out", [BS, OUT_DIM], f32, isOutput=True)

    CHUNK_IT = N_IT // 4           # iterations per resident xt chunk
    CHUNK_ROWS = CHUNK_IT * ROWS_PER_IT

    with tile.TileContext(nc) as tc:
        with (
            tc.tile_pool(name="const", bufs=1) as const_pool,
            tc.tile_pool(name="xin", bufs=4) as x_pool,
            tc.tile_pool(name="outp", bufs=6) as out_pool,
            tc.tile_pool(name="ps", bufs=2, space="PSUM") as psum_pool,
        ):
            rhs_t = const_pool.tile([K, OUT_DIM], bf16)
            # all inputs preload through GpSimd SWDGE at kernel start: the
            # Sync HWDGE ring stays store-only (FIFO per engine), and the
            # SDMA engines aren't shared with loads during the store stream
            nc.gpsimd.dma_start(out=rhs_t[:], in_=rhs[:, :])
            xt_chunks = []
            for cidx in range(4):
                xc = x_pool.tile([K, CHUNK_ROWS], bf16, tag="xc")
                nc.gpsimd.dma_start(
                    out=xc[:],
                    in_=xt[:, cidx * CHUNK_ROWS:(cidx + 1) * CHUNK_ROWS],
                )
                xt_chunks.append(xc)

            def lhs_slice(it, w):
                cidx, coff = divmod(it, CHUNK_IT)
                base = coff * ROWS_PER_IT
                return xt_chunks[cidx][:, base + w * P:base + (w + 1) * P]

            def dram_view(it):
                # host permutation makes partition p own 8 consecutive rows
                return out[it * ROWS_PER_IT:(it + 1) * ROWS_PER_IT, :].rearrange(
                    "(p w) o -> p (w o)", w=W
                )

            def steady_iter(it):
                ps = psum_pool.tile([P, FREE], f32, tag="ps")
                ot = out_pool.tile([P, FREE], f32, tag="ot")
                for w in range(W):
                    # theta_c[p, o] accumulated in fp32 over 15 bf16 products
                    nc.tensor.matmul(
                        ps[:, w * OUT_DIM:(w + 1) * OUT_DIM],
                        lhs_slice(it, w),
                        rhs_t[:],
                        start=True,
                        stop=True,
                    )
                nc.scalar.activation(
                    ot[:], ps[:], mybir.ActivationFunctionType.Sin
                )
                nc.sync.dma_start(out=dram_view(it), in_=ot[:])

            def ramp_iter(it):
                # W=2 sub-steps: primes (it=0) / drains (it=N_IT-1) the store
                # pipeline ~4 us faster than a full 2048-wide tile
                dview = dram_view(it)
                for j in range(W // 2):
                    ps = psum_pool.tile([P, 2 * OUT_DIM], f32, tag="ps")
                    ot = out_pool.tile([P, 2 * OUT_DIM], f32, tag="ot")
                    for k in range(2):
                        nc.tensor.matmul(
                            ps[:, k * OUT_DIM:(k + 1) * OUT_DIM],
                            lhs_slice(it, 2 * j + k),
                            rhs_t[:],
                            start=True,
                            stop=True,
                        )
                    nc.scalar.activation(
                        ot[:], ps[:], mybir.ActivationFunctionType.Sin
                    )
                    nc.sync.dma_start(
                        out=dview[:, j * 2 * OUT_DIM:(j + 1) * 2 * OUT_DIM],
                        in_=ot[:],
                    )

            ramp_iter(0)
            for it in range(1, N_IT - 1):
                steady_iter(it)
            ramp_iter(N_IT - 1)
    nc.compile()
    return nc


def _build_inputs(x, h, delta):
    """Per-core input maps: K=15 bf16-split lhsT (batch-permuted) + rhs."""
    # 3-term bf16 splits of the h columns and of delta + pi/2 (in float64)
    p1, p2, p3 = _split3(h[:, 0].astype(np.float64))
    q1, q2, q3 = _split3(h[:, 1].astype(np.float64))
    d1, d2, d3 = _split3(delta.astype(np.float64) + np.pi / 2)
    rhs = np.stack(
        [p1, p2, p3, p1, p2, p1, q1, q2, q3, q1, q2, q1, d1, d2, d3]
    ).astype(BF16)

    in_maps = []
    ones = np.ones(BS, BF16)
    for c in range(N_CORES):
        xs = x[c * BS:(c + 1) * BS]
        # permute within each 1024-row block: col w*128+p <- row p*8+w
        xp = (
            xs.reshape(N_IT, P, W, IN_DIM)
            .transpose(0, 2, 1, 3)
            .reshape(BS, IN_DIM)
        )
        a0, b0, c0 = _split3(xp[:, 0].astype(np.float64))
        a1, b1, c1 = _split3(xp[:, 1].astype(np.float64))
        # row k of lhsT pairs with row k of rhs; delta terms last so the
        # systolic partial sums stay small until the pi/2-sized terms enter
        xt = np.stack(
            [a0, a0, a0, b0, b0, c0, a1, a1, a1, b1, b1, c1, ones, ones, ones]
        ).astype(BF16)
        in_maps.append({"xt": xt, "rhs": rhs})
    return in_maps


def _eta_host(x64_u, x64_v, h, delta):
    """mean_b cos(theta)/sin(theta) per column via moment expansion, then eta.

    theta_{b,o} = u_b p_o + v_b q_o + d_o.  E_b[theta^n] is a polynomial in
    the power sums S_ij = sum_b u^i v^j, so the sin/cos means come from a
    Taylor series evaluated in float64 on the host.
    """
    n_ord = 20
    bsz = x64_u.shape[0]
    p = h[:, 0].astype(np.float64)
    q = h[:, 1].astype(np.float64)
    d = delta.astype(np.float64)

    # Power sums S[i, j] = sum_b u^i v^j  (BLAS dgemm does the heavy part).
    U = np.empty((bsz, n_ord + 1), np.float64)
    V = np.empty((bsz, n_ord + 1), np.float64)
    U[:, 0] = 1.0
    V[:, 0] = 1.0
    for k in range(1, n_ord + 1):
        U[:, k] = U[:, k - 1] * x64_u
        V[:, k] = V[:, k - 1] * x64_v
    S = U.T @ V

    # Mw[m, o] = E_b[(u p_o + v q_o)^m]
    Mw = np.zeros((n_ord + 1, OUT_DIM), np.float64)
    for m in range(n_ord + 1):
        acc = np.zeros(OUT_DIM, np.float64)
        for i in range(m + 1):
            acc += math.comb(m, i) * S[i, m - i] * p**i * q**(m - i)
        Mw[m] = acc / bsz

    # E[theta^n] = sum_m C(n, m) Mw[m] d^(n-m); fold into sin/cos series
    mean_r = np.zeros(OUT_DIM, np.float64)
    mean_i = np.zeros(OUT_DIM, np.float64)
    for n in range(n_ord + 1):
        eth = np.zeros(OUT_DIM, np.float64)
        for m in range(n + 1):
            eth += math.comb(n, m) * Mw[m] * d ** (n - m)
        term = eth / math.factorial(n)
        if n % 2 == 0:
            mean_r += term * (-1) ** (n // 2)
        else:
            mean_i += term * (-1) ** ((n - 1) // 2)

    eta = np.sum(np.arctan2(mean_i, mean_r)) / np.pi
    return np.float32(eta)


def _eta_host_direct(x64_u, x64_v, h, delta):
    """Fallback: direct chunked float64 evaluation of the sin/cos means."""
    p = h[:, 0].astype(np.float64)
    q = h[:, 1].astype(np.float64)
    d = delta.astype(np.float64)
    sum_r = np.zeros(OUT_DIM, np.float64)
    sum_i = np.zeros(OUT_DIM, np.float64)
    chunk = 16384
    for s in range(0, x64_u.shape[0], chunk):
        th = np.outer(x64_u[s:s + chunk], p) + np.outer(x64_v[s:s + chunk], q) + d
        sum_r += np.cos(th).sum(axis=0)
        sum_i += np.sin(th).sum(axis=0)
    bsz = x64_u.shape[0]
    eta = np.sum(np.arctan2(sum_i / bsz, sum_r / bsz)) / np.pi
    return np.float32(eta)


def kernel(x, h, delta):
    global LAST_RESULTS
    x = np.asarray(x, dtype=np.float32)
    h = np.asarray(h, dtype=np.float32)
    delta = np.asarray(delta, dtype=np.float32)

    if "nc" not in _cache:
        _cache["nc"] = _build_nc()
    nc = _cache["nc"]

    in_maps = _build_inputs(x, h, delta)
    res = run_bass_kernel_spmd(nc, in_maps, core_ids=list(range(N_CORES)))
    LAST_RESULTS = res
    # the DMA view "(p w) o" already writes rows back in natural batch order
    out_real = np.concatenate(
        [res.results[c]["out"] for c in range(N_CORES)], axis=0
    )

    u = x[:, 0].astype(np.float64)
    v = x[:, 1].astype(np.float64)
    # |theta| bound decides whether the Taylor/moment path is safe.
    theta_bound = (
        np.abs(u).max() * np.abs(h[:, 0]).max()
        + np.abs(v).max() * np.abs(h[:, 1]).max()
        + np.abs(delta).max()
    )
    if theta_bound < 2.0:
        eta = _eta_host(u, v, h, delta)
    else:
        eta = _eta_host_direct(u, v, h, delta)
    return out_real, eta


# revision 11
# speedup vs baseline: 1.1048x; 1.1048x over previous
"""Trainium2 Bass kernel for nn_FDCKernel (fused phase / cos + eta reduction).

Reference computation:
    theta    = x @ h.T + delta          # [B, 256], B = 262144
    out_real = cos(theta)               # returned, 256 MiB -> memory bound
    eta      = sum(arctan2(mean_b sin(theta), mean_b cos(theta))) / pi

Strategy:
  * Pure data parallelism over the batch across 8 NeuronCores.
  * Per core, theta_c = theta + pi/2 comes from ONE PE matmul per 128-row
    tile.  fp32 matmuls on TRN2 run as 2x(LDWEIGHTS+MATMUL) LOW/HIGH passes
    (~854 ns each), so instead the K=3 fp32 contraction is decomposed into a
    K=15 all-bf16 contraction (3-term bf16 splits of x, h and delta+pi/2,
    keeping all cross products >= 2^-27): single-pass bf16 matmul + FWL
    weight loads, ~4-6x faster on PE, while PSUM still accumulates in fp32
    so theta keeps ~1e-7 accuracy.
  * ACT engine turns [128, 2048] PSUM tiles into cos via Sin(theta + pi/2)
    in one transcendental pass.
  * The host pre-permutes the batch within every 1024-row block (col w*128+p
    holds row p*8+w) so each SBUF partition owns 8 consecutive output rows:
    DMA stores are 1 MiB per dma_start with 8 KiB contiguous descriptors.
  * eta needs only the per-column MEANS of sin/cos.  theta = u*p + v*q + d
    with |theta| << 1, so mean_b cos/sin expand exactly into polynomial
    moments S_ij = sum_b u^i v^j of the tiny [B,2] input x, evaluated on the
    host in float64 (Taylor to order 20) - no device sin pass, no collective.
"""

import math
import sys

import ml_dtypes
import numpy as np

sys.path.insert(0, "/opt/trn_rl_repo")

import concourse.bass as bass
import concourse.tile as tile
from concourse import bacc, mybir
from concourse.bass_utils import run_bass_kernel_spmd

B, IN_DIM, OUT_DIM = 262144, 2, 256
N_CORES = 8
BS = B // N_CORES            # 32768 rows per core
P = 128                      # batch rows per partition tile
W = 8                        # batch subtiles per PSUM tile (free dim = W*256 = 2048)
ROWS_PER_IT = P * W          # 1024
N_IT = BS // ROWS_PER_IT     # 32
FREE = W * OUT_DIM           # 2048
K = 15                       # bf16 contraction depth (see _build_lhsT/_build_rhs)

BF16 = ml_dtypes.bfloat16

_cache = {}
LAST_RESULTS = None          # BassKernelResults of the most recent run (for profiling)


def _split3(v64):
    """v (float64/32) ~= t1+t2+t3 with bf16 terms; residual <= 2^-27 |v|."""
    t1 = v64.astype(BF16)
    r1 = (v64 - t1.astype(np.float64)).astype(np.float64)
    t2 = r1.astype(BF16)
    r2 = r1 - t2.astype(np.float64)
    t3 = r2.astype(BF16)
    return t1, t2, t3


def _build_nc():
    f32 = mybir.dt.float32
    bf16 = mybir.dt.bfloat16
    # Bacc (not raw Bass): its compile() runs the TRN2 legalization passes —
    # move_matmul_waits_to_ldweights + generate_event_semaphores (1-wait limit).
    nc = bacc.Bacc(
        "TRN2", target_bir_lowering=False, debug=False, num_devices=N_CORES
    )
    xt = nc.declare_dram_parameter("xt", [K, BS], bf16, isOutput=False)
    rhs = nc.declare_dram_parameter("rhs", [K, OUT_DIM], bf16, isOutput=False)
    out = nc.declare_dram_parameter("out", [BS, OUT_DIM], f32, isOutput=True)

    CHUNK_IT = N_IT // 4           # iterations per resident xt chunk
    CHUNK_ROWS = CHUNK_IT * ROWS_PER_IT

    with tile.TileContext(nc) as tc:
        with (
            tc.tile_pool(name="const", bufs=1) as const_pool,
            tc.tile_pool(name="xin", bufs=4) as x_pool,
            tc.tile_pool(name="outp", bufs=6) as out_pool,
            tc.tile_pool(name="ps", bufs=2, space="PSUM") as psum_pool,
        ):
            rhs_t = const_pool.tile([K, OUT_DIM], bf16)
            # preload all inputs before the store stream exists: rhs + the
            # first chunk ride the (still empty) Sync HWDGE ring so the first
            # matmuls start right after the engine preamble; later chunks go
            # through GpSimd SWDGE, whose ~8 us preamble still lands them
            # long before iteration 8 needs chunk 1
            nc.sync.dma_start(out=rhs_t[:], in_=rhs[:, :])
            xt_chunks = []
            for cidx in range(4):
                xc = x_pool.tile([K, CHUNK_ROWS], bf16, tag="xc")
                eng = nc.sync if cidx == 0 else nc.gpsimd
                eng.dma_start(
                    out=xc[:],
                    in_=xt[:, cidx * CHUNK_ROWS:(cidx + 1) * CHUNK_ROWS],
                )
                xt_chunks.append(xc)

            def lhs_slice(it, w):
                cidx, coff = divmod(it, CHUNK_IT)
                base = coff * ROWS_PER_IT
                return xt_chunks[cidx][:, base + w * P:base + (w + 1) * P]

            def dram_view(it):
                # host permutation makes partition p own 8 consecutive rows
                return out[it * ROWS_PER_IT:(it + 1) * ROWS_PER_IT, :].rearrange(
                    "(p w) o -> p (w o)", w=W
                )

            def steady_iter(it):
                ps = psum_pool.tile([P, FREE], f32, tag="ps")
                ot = out_pool.tile([P, FREE], f32, tag="ot")
                for w in range(W):
                    # theta_c[p, o] accumulated in fp32 over 15 bf16 products
                    nc.tensor.matmul(
                        ps[:, w * OUT_DIM:(w + 1) * OUT_DIM],
                        lhs_slice(it, w),
                        rhs_t[:],
                        start=True,
                        stop=True,
                    )
                nc.scalar.activation(
                    ot[:], ps[:], mybir.ActivationFunctionType.Sin
                )
                nc.sync.dma_start(out=dram_view(it), in_=ot[:])

            def ramp_iter(it):
                # W=2 sub-steps: primes (it=0) / drains (it=N_IT-1) the store
                # pipeline ~4 us faster than a full 2048-wide tile
                dview = dram_view(it)
                for j in range(W // 2):
                    ps = psum_pool.tile([P, 2 * OUT_DIM], f32, tag="ps")
                    ot = out_pool.tile([P, 2 * OUT_DIM], f32, tag="ot")
                    for k in range(2):
                        nc.tensor.matmul(
                            ps[:, k * OUT_DIM:(k + 1) * OUT_DIM],
                            lhs_slice(it, 2 * j + k),
                            rhs_t[:],
                            start=True,
                            stop=True,
                        )
                    nc.scalar.activation(
                        ot[:], ps[:], mybir.ActivationFunctionType.Sin
                    )
                    nc.sync.dma_start(
                        out=dview[:, j * 2 * OUT_DIM:(j + 1) * 2 * OUT_DIM],
                        in_=ot[:],
                    )

            ramp_iter(0)
            for it in range(1, N_IT - 1):
                steady_iter(it)
            ramp_iter(N_IT - 1)
    nc.compile()
    return nc


def _build_inputs(x, h, delta):
    """Per-core input maps: K=15 bf16-split lhsT (batch-permuted) + rhs."""
    # 3-term bf16 splits of the h columns and of delta + pi/2 (in float64)
    p1, p2, p3 = _split3(h[:, 0].astype(np.float64))
    q1, q2, q3 = _split3(h[:, 1].astype(np.float64))
    d1, d2, d3 = _split3(delta.astype(np.float64) + np.pi / 2)
    rhs = np.stack(
        [p1, p2, p3, p1, p2, p1, q1, q2, q3, q1, q2, q1, d1, d2, d3]
    ).astype(BF16)

    in_maps = []
    ones = np.ones(BS, BF16)
    for c in range(N_CORES):
        xs = x[c * BS:(c + 1) * BS]
        # permute within each 1024-row block: col w*128+p <- row p*8+w
        xp = (
            xs.reshape(N_IT, P, W, IN_DIM)
            .transpose(0, 2, 1, 3)
            .reshape(BS, IN_DIM)
        )
        a0, b0, c0 = _split3(xp[:, 0].astype(np.float64))
        a1, b1, c1 = _split3(xp[:, 1].astype(np.float64))
        # row k of lhsT pairs with row k of rhs; delta terms last so the
        # systolic partial sums stay small until the pi/2-sized terms enter
        xt = np.stack(
            [a0, a0, a0, b0, b0, c0, a1, a1, a1, b1, b1, c1, ones, ones, ones]
        ).astype(BF16)
        in_maps.append({"xt": xt, "rhs": rhs})
    return in_maps


def _eta_host(x64_u, x64_v, h, delta):
    """mean_b cos(theta)/sin(theta) per column via moment expansion, then eta.

    theta_{b,o} = u_b p_o + v_b q_o + d_o.  E_b[theta^n] is a polynomial in
    the power sums S_ij = sum_b u^i v^j, so the sin/cos means come from a
    Taylor series evaluated in float64 on the host.
    """
    n_ord = 20
    bsz = x64_u.shape[0]
    p = h[:, 0].astype(np.float64)
    q = h[:, 1].astype(np.float64)
    d = delta.astype(np.float64)

    # Power sums S[i, j] = sum_b u^i v^j  (BLAS dgemm does the heavy part).
    U = np.empty((bsz, n_ord + 1), np.float64)
    V = np.empty((bsz, n_ord + 1), np.float64)
    U[:, 0] = 1.0
    V[:, 0] = 1.0
    for k in range(1, n_ord + 1):
        U[:, k] = U[:, k - 1] * x64_u
        V[:, k] = V[:, k - 1] * x64_v
    S = U.T @ V

    # Mw[m, o] = E_b[(u p_o + v q_o)^m]
    Mw = np.zeros((n_ord + 1, OUT_DIM), np.float64)
    for m in range(n_ord + 1):
        acc = np.zeros(OUT_DIM, np.float64)
        for i in range(m + 1):
            acc += math.comb(m, i) * S[i, m - i] * p**i * q**(m - i)
        Mw[m] = acc / bsz

    # E[theta^n] = sum_m C(n, m) Mw[m] d^(n-m); fold into sin/cos series
    mean_r = np.zeros(OUT_DIM, np.float64)
    mean_i = np.zeros(OUT_DIM, np.float64)
    for n in range(n_ord + 1):
        eth = np.zeros(OUT_DIM, np.float64)
        for m in range(n + 1):
            eth += math.comb(n, m) * Mw[m] * d ** (n - m)
        term = eth / math.factorial(n)
        if n % 2 == 0:
            mean_r += term * (-1) ** (n // 2)
        else:
            mean_i += term * (-1) ** ((n - 1) // 2)

    eta = np.sum(np.arctan2(mean_i, mean_r)) / np.pi
    return np.float32(eta)


def _eta_host_direct(x64_u, x64_v, h, delta):
    """Fallback: direct chunked float64 evaluation of the sin/cos means."""
    p = h[:, 0].astype(np.float64)
    q = h[:, 1].astype(np.float64)
    d = delta.astype(np.float64)
    sum_r = np.zeros(OUT_DIM, np.float64)
    sum_i = np.zeros(OUT_DIM, np.float64)
    chunk = 16384
    for s in range(0, x64_u.shape[0], chunk):
        th = np.outer(x64_u[s:s + chunk], p) + np.outer(x64_v[s:s + chunk], q) + d
        sum_r += np.cos(th).sum(axis=0)
        sum_i += np.sin(th).sum(axis=0)
    bsz = x64_u.shape[0]
    eta = np.sum(np.arctan2(sum_i / bsz, sum_r / bsz)) / np.pi
    return np.float32(eta)


def kernel(x, h, delta):
    global LAST_RESULTS
    x = np.asarray(x, dtype=np.float32)
    h = np.asarray(h, dtype=np.float32)
    delta = np.asarray(delta, dtype=np.float32)

    if "nc" not in _cache:
        _cache["nc"] = _build_nc()
    nc = _cache["nc"]

    in_maps = _build_inputs(x, h, delta)
    res = run_bass_kernel_spmd(nc, in_maps, core_ids=list(range(N_CORES)))
    LAST_RESULTS = res
    # the DMA view "(p w) o" already writes rows back in natural batch order
    out_real = np.concatenate(
        [res.results[c]["out"] for c in range(N_CORES)], axis=0
    )

    u = x[:, 0].astype(np.float64)
    v = x[:, 1].astype(np.float64)
    # |theta| bound decides whether the Taylor/moment path is safe.
    theta_bound = (
        np.abs(u).max() * np.abs(h[:, 0]).max()
        + np.abs(v).max() * np.abs(h[:, 1]).max()
        + np.abs(delta).max()
    )
    if theta_bound < 2.0:
        eta = _eta_host(u, v, h, delta)
    else:
        eta = _eta_host_direct(u, v, h, delta)
    return out_real, eta


# revision 25
# speedup vs baseline: 1.1695x; 1.0586x over previous
"""Trainium2 Bass kernel for nn_FDCKernel (fused phase / cos + eta reduction).

Reference computation:
    theta    = x @ h.T + delta          # [B, 256], B = 262144
    out_real = cos(theta)               # returned, 256 MiB -> memory bound
    eta      = sum(arctan2(mean_b sin(theta), mean_b cos(theta))) / pi

Strategy:
  * Pure data parallelism over the batch across 8 NeuronCores.
  * Per core, theta_c = theta + pi/2 comes from ONE PE matmul per 128-row
    tile.  fp32 matmuls on TRN2 run as 2x(LDWEIGHTS+MATMUL) LOW/HIGH passes
    (~854 ns each), so instead the K=3 fp32 contraction is decomposed into a
    K=15 all-bf16 contraction (3-term bf16 splits of x, h and delta+pi/2,
    keeping all cross products >= 2^-27): single-pass bf16 matmul + FWL
    weight loads, ~4-6x faster on PE, while PSUM still accumulates in fp32
    so theta keeps ~1e-7 accuracy.
  * ACT engine turns [128, 2048] PSUM tiles into cos via Sin(theta + pi/2)
    in one transcendental pass.
  * The host pre-permutes the batch within every 1024-row block (col w*128+p
    holds row p*8+w) so each SBUF partition owns 8 consecutive output rows:
    DMA stores are 1 MiB per dma_start with 8 KiB contiguous descriptors.
  * eta needs only the per-column MEANS of sin/cos.  theta = u*p + v*q + d
    with |theta| << 1, so mean_b cos/sin expand exactly into polynomial
    moments S_ij = sum_b u^i v^j of the tiny [B,2] input x, evaluated on the
    host in float64 (Taylor to order 20) - no device sin pass, no collective.
"""

import math
import os
import sys
import time

import ml_dtypes
import numpy as np

sys.path.insert(0, "/opt/trn_rl_repo")

import concourse.bass as bass
import concourse.tile as tile
from concourse import bacc, mybir
from concourse.bass_utils import run_bass_kernel_spmd

# bass_utils imports antenv.axon_hooks when BASS_TRACE is set; some images
# lack that module, which would turn an optional trace into a hard crash.
try:
    import antenv.axon_hooks  # noqa: F401
except Exception:
    import types

    import antenv

    _ah = types.ModuleType("antenv.axon_hooks")
    _ah._hook = None
    _ah.set_axon_ntff_profile_hook = lambda h: setattr(_ah, "_hook", h)
    _ah.get_axon_ntff_profile_hook = lambda: _ah._hook
    sys.modules["antenv.axon_hooks"] = _ah
    antenv.axon_hooks = _ah

B, IN_DIM, OUT_DIM = 262144, 2, 256
N_CORES = 8
BS = B // N_CORES            # 32768 rows per core
P = 128                      # batch rows per partition tile
W = 8                        # batch subtiles per PSUM tile (free dim = W*256 = 2048)
ROWS_PER_IT = P * W          # 1024
N_IT = BS // ROWS_PER_IT     # 32
FREE = W * OUT_DIM           # 2048
K = 15                       # bf16 contraction depth (see _build_inputs)
KP = 16                      # padded to 16 (zero row) so lhsT packs to 128 partitions

BF16 = ml_dtypes.bfloat16

_cache = {}
LAST_RESULTS = None          # BassKernelResults of the most recent run (for profiling)


def _split3(v64):
    """v (float64/32) ~= t1+t2+t3 with bf16 terms; residual <= 2^-27 |v|."""
    t1 = v64.astype(BF16)
    r1 = (v64 - t1.astype(np.float64)).astype(np.float64)
    t2 = r1.astype(BF16)
    r2 = r1 - t2.astype(np.float64)
    t3 = r2.astype(BF16)
    return t1, t2, t3


def _build_nc():
    f32 = mybir.dt.float32
    bf16 = mybir.dt.bfloat16
    # Bacc (not raw Bass): its compile() runs the TRN2 legalization passes —
    # move_matmul_waits_to_ldweights + generate_event_semaphores (1-wait limit).
    nc = bacc.Bacc(
        "TRN2", target_bir_lowering=False, debug=False, num_devices=N_CORES
    )
    xt = nc.declare_dram_parameter("xt", [K, BS], bf16, isOutput=False)
    rhs = nc.declare_dram_parameter("rhs", [K, OUT_DIM], bf16, isOutput=False)
    out = nc.declare_dram_parameter("out", [BS, OUT_DIM], f32, isOutput=True)

    with tile.TileContext(nc) as tc:
        with (
            tc.tile_pool(name="const", bufs=1) as const_pool,
            tc.tile_pool(name="xin", bufs=6) as x_pool,
            tc.tile_pool(name="outp", bufs=8) as out_pool,
            tc.tile_pool(name="ps", bufs=2, space="PSUM") as psum_pool,
        ):
            rhs_t = const_pool.tile([K, OUT_DIM], bf16)
            # rhs rides the (still empty at t=0) Sync HWDGE ring; the 30 KiB
            # per-iteration lhsT loads go through GpSimd SWDGE so they never
            # queue behind 1 MiB stores on the Sync ring.  A 15-partition
            # tile only engages ~2 SDMA engines (~0.6 us per load), which the
            # 6-deep lookahead hides completely.
            nc.sync.dma_start(out=rhs_t[:], in_=rhs[:, :])

            def load_iter(it):
                xt_t = x_pool.tile([K, ROWS_PER_IT], bf16, tag="xt")
                nc.gpsimd.dma_start(
                    out=xt_t[:],
                    in_=xt[:, it * ROWS_PER_IT:(it + 1) * ROWS_PER_IT],
                )
                return xt_t

            def lhs_slice(xt_t, w):
                return xt_t[:, w * P:(w + 1) * P]

            def dram_view(it):
                # host permutation makes partition p own 8 consecutive rows
                return out[it * ROWS_PER_IT:(it + 1) * ROWS_PER_IT, :].rearrange(
                    "(p w) o -> p (w o)", w=W
                )

            def steady_iter(it):
                xt_t = load_iter(it)
                ps = psum_pool.tile([P, FREE], f32, tag="ps")
                ot = out_pool.tile([P, FREE], f32, tag="ot")
                for w in range(W):
                    # theta_c[p, o] accumulated in fp32 over 15 bf16 products
                    nc.tensor.matmul(
                        ps[:, w * OUT_DIM:(w + 1) * OUT_DIM],
                        lhs_slice(xt_t, w),
                        rhs_t[:],
                        start=True,
                        stop=True,
                    )
                nc.scalar.activation(
                    ot[:], ps[:], mybir.ActivationFunctionType.Sin
                )
                nc.sync.dma_start(out=dram_view(it), in_=ot[:])

            def ramp_iter(it):
                # W=2 sub-steps: primes (it=0) / drains (it=N_IT-1) the store
                # pipeline ~4 us faster than a full 2048-wide tile
                xt_t = load_iter(it)
                dview = dram_view(it)
                for j in range(W // 2):
                    ps = psum_pool.tile([P, 2 * OUT_DIM], f32, tag="ps")
                    ot = out_pool.tile([P, 2 * OUT_DIM], f32, tag="ot")
                    for k in range(2):
                        nc.tensor.matmul(
                            ps[:, k * OUT_DIM:(k + 1) * OUT_DIM],
                            lhs_slice(xt_t, 2 * j + k),
                            rhs_t[:],
                            start=True,
                            stop=True,
                        )
                    nc.scalar.activation(
                        ot[:], ps[:], mybir.ActivationFunctionType.Sin
                    )
                    nc.sync.dma_start(
                        out=dview[:, j * 2 * OUT_DIM:(j + 1) * 2 * OUT_DIM],
                        in_=ot[:],
                    )

            if os.environ.get("FDC_NO_RAMP"):
                for it in range(N_IT):
                    steady_iter(it)
            else:
                ramp_iter(0)
                for it in range(1, N_IT - 1):
                    steady_iter(it)
                ramp_iter(N_IT - 1)
    nc.compile()
    return nc


def _build_inputs(x, h, delta):
    """Per-core input maps: K=15 bf16-split lhsT (batch-permuted) + rhs."""
    # 3-term bf16 splits of the h columns and of delta + pi/2 (in float64)
    p1, p2, p3 = _split3(h[:, 0].astype(np.float64))
    q1, q2, q3 = _split3(h[:, 1].astype(np.float64))
    d1, d2, d3 = _split3(delta.astype(np.float64) + np.pi / 2)
    rhs = np.stack(
        [p1, p2, p3, p1, p2, p1, q1, q2, q3, q1, q2, q1, d1, d2, d3]
    ).astype(BF16)

    in_maps = []
    ones = np.ones(BS, BF16)
    for c in range(N_CORES):
        xs = x[c * BS:(c + 1) * BS]
        # permute within each 1024-row block: col w*128+p <- row p*8+w
        xp = (
            xs.reshape(N_IT, P, W, IN_DIM)
            .transpose(0, 2, 1, 3)
            .reshape(BS, IN_DIM)
        )
        a0, b0, c0 = _split3(xp[:, 0].astype(np.float64))
        a1, b1, c1 = _split3(xp[:, 1].astype(np.float64))
        # row k of lhsT pairs with row k of rhs; delta terms last so the
        # systolic partial sums stay small until the pi/2-sized terms enter
        xk = np.stack(
            [a0, a0, a0, b0, b0, c0, a1, a1, a1, b1, b1, c1, ones, ones, ones]
        ).astype(BF16)                              # [15, BS], col = it*1024+w*128+p
        in_maps.append({"xt": xk, "rhs": rhs})
    return in_maps


def _eta_host(x64_u, x64_v, h, delta):
    """mean_b cos(theta)/sin(theta) per column via moment expansion, then eta.

    theta_{b,o} = u_b p_o + v_b q_o + d_o.  E_b[theta^n] is a polynomial in
    the power sums S_ij = sum_b u^i v^j, so the sin/cos means come from a
    Taylor series evaluated in float64 on the host.
    """
    n_ord = 20
    bsz = x64_u.shape[0]
    p = h[:, 0].astype(np.float64)
    q = h[:, 1].astype(np.float64)
    d = delta.astype(np.float64)

    # Power sums S[i, j] = sum_b u^i v^j  (BLAS dgemm does the heavy part).
    U = np.empty((bsz, n_ord + 1), np.float64)
    V = np.empty((bsz, n_ord + 1), np.float64)
    U[:, 0] = 1.0
    V[:, 0] = 1.0
    for k in range(1, n_ord + 1):
        U[:, k] = U[:, k - 1] * x64_u
        V[:, k] = V[:, k - 1] * x64_v
    S = U.T @ V

    # Mw[m, o] = E_b[(u p_o + v q_o)^m]
    Mw = np.zeros((n_ord + 1, OUT_DIM), np.float64)
    for m in range(n_ord + 1):
        acc = np.zeros(OUT_DIM, np.float64)
        for i in range(m + 1):
            acc += math.comb(m, i) * S[i, m - i] * p**i * q**(m - i)
        Mw[m] = acc / bsz

    # E[theta^n] = sum_m C(n, m) Mw[m] d^(n-m); fold into sin/cos series
    mean_r = np.zeros(OUT_DIM, np.float64)
    mean_i = np.zeros(OUT_DIM, np.float64)
    for n in range(n_ord + 1):
        eth = np.zeros(OUT_DIM, np.float64)
        for m in range(n + 1):
            eth += math.comb(n, m) * Mw[m] * d ** (n - m)
        term = eth / math.factorial(n)
        if n % 2 == 0:
            mean_r += term * (-1) ** (n // 2)
        else:
            mean_i += term * (-1) ** ((n - 1) // 2)

    eta = np.sum(np.arctan2(mean_i, mean_r)) / np.pi
    return np.float32(eta)


def _eta_host_direct(x64_u, x64_v, h, delta):
    """Fallback: direct chunked float64 evaluation of the sin/cos means."""
    p = h[:, 0].astype(np.float64)
    q = h[:, 1].astype(np.float64)
    d = delta.astype(np.float64)
    sum_r = np.zeros(OUT_DIM, np.float64)
    sum_i = np.zeros(OUT_DIM, np.float64)
    chunk = 16384
    for s in range(0, x64_u.shape[0], chunk):
        th = np.outer(x64_u[s:s + chunk], p) + np.outer(x64_v[s:s + chunk], q) + d
        sum_r += np.cos(th).sum(axis=0)
        sum_i += np.sin(th).sum(axis=0)
    bsz = x64_u.shape[0]
    eta = np.sum(np.arctan2(sum_i / bsz, sum_r / bsz)) / np.pi
    return np.float32(eta)


def kernel(x, h, delta):
    global LAST_RESULTS
    x = np.asarray(x, dtype=np.float32)
    h = np.asarray(h, dtype=np.float32)
    delta = np.asarray(delta, dtype=np.float32)

    if "nc" not in _cache:
        _cache["nc"] = _build_nc()
    nc = _cache["nc"]

    in_maps = _build_inputs(x, h, delta)
    try:
        res = run_bass_kernel_spmd(nc, in_maps, core_ids=list(range(N_CORES)))
    except Exception:
        # transient NRT exec-unit errors have been observed; retry once
        time.sleep(2.0)
        res = run_bass_kernel_spmd(nc, in_maps, core_ids=list(range(N_CORES)))
    LAST_RESULTS = res
    # the DMA view "(p w) o" already writes rows back in natural batch order
    out_real = np.concatenate(
        [res.results[c]["out"] for c in range(N_CORES)], axis=0
    )

    u = x[:, 0].astype(np.float64)
    v = x[:, 1].astype(np.float64)
    # |theta| bound decides whether the Taylor/moment path is safe.
    theta_bound = (
        np.abs(u).max() * np.abs(h[:, 0]).max()
        + np.abs(v).max() * np.abs(h[:, 1]).max()
        + np.abs(delta).max()
    )
    if theta_bound < 2.0:
        eta = _eta_host(u, v, h, delta)
    else:
        eta = _eta_host_direct(u, v, h, delta)
    return out_real, eta
